# revision 29
# baseline (speedup 1.0000x reference)
"""Trainium2 Bass kernel for nn_ConvAttention_34600256537137.

Math notes (validated against the reference):
  qkv = 1x1conv(x, w1)+b1 -> Q,K,V;  score = conv5x5(Q_s)+conv5x5(K_t)+b2;
  attn = softmax_t(score);  out = einsum(attn, V).
  Softmax over t is shift-invariant, so the Q-half of the score (constant in
  t), b2, and the K-path bias all cancel.  The computation collapses to:
    weff[ci,dy,dx] = sum_c w1K[c,ci] * w2K[c,dy,dx]        (host, tiny)
    sK[b,t,h,w]    = conv5x5_reflect(x[b,:,:,:,t], weff)
    e = exp(sK);  den = sum_t e
    out[b,o,h,w,s] = (sum_{ci,t} w1V[o,ci] * e * x) / den + b1V[o]
  (s-independent; normalization folded to the end; bias + S-broadcast on host)

Sharding: 8 cores = (b in {0,1}) x (4 chunks of 8 rows of H).

Perf structure (final; bf16 datapath, row-PAIR conv folding; ~32us vs 37.8us
baseline):
  - conv contraction folds (ci, row-parity rp): K=128, stationary columns
    (po, dx, par) with weff3[(ci,rp), (po,dx,par)] = weff[ci, 2po+rp-par, dx]
    (out-of-range dy -> 0).  12 matmuls (6 slab row-pairs x 2 t-halves) of
    288 cols -- same PE cycles as a dy-class scheme but only THREE tap
    classes (po) survive to the partition-transpose bounce.
  - plane order (po, dx, par) makes the td par-stride == PLANE == 64*36, so
    the (par,t,q0) gather dims merge: ONE 3-dim gather per po (3 gathers of
    640 descriptors instead of 10 of 320).
  - HBM loads are latency-chain bound (~8 descs/SDMA engine, ~400-650ns per
    descriptor round-trip, no pipelining): slabA = [weff3 | pair0] and
    slabB = [pairs 1-5] ride the two HWDGE rings as one chain each; the
    consts (hsel|hmask|esel) and w1vr queue behind slabA; xt rides SWDGE
    (gpsimd) using otherwise-idle SDMA capacity.
  - PE warm-up matmul chains (junk data -> scratch PSUM bank) hold the
    tensor engine p-state up before the conv (measured 444 -> ~284ns per
    288-col matmul) and again before the den/eb burst.
  - PSUM evacuation copies split into halves on scalar+vector so the copy
    rate matches the warmed matmul rate; td write po issues as soon as its
    4-pair window of copies lands (w0 after 8 of 12 matmuls).
  - per-po partial tap reduces hide behind the gather DMA drains; only the
    last po's reduce + 3-way add + exp sit on the critical path.
  - softmax denominator via indicator-matmul (hsel) on PE; e is replicated
    to the (ci8,t)-partition layout ON-CHIP: a mask-multiply expands e16 to
    the partition-diagonal [128, (h,w)] tile and one esel indicator matmul
    emits eb[(rep,t), (h,w)] in PSUM -- no DRAM round trip.
  - xattn = xt*eb in four chunks on vector; the V matmuls pipeline behind
    it (effective ~213ns/256-col matmul once pipelined); den-mm/recip/dn
    trail off the critical path; output is emitted bf16 (error budget 2e-2
    >> bf16 rounding; measured rel err ~5.5e-3) in two partition-halves on
    both HWDGE rings in parallel; the host applies 1/den (dn) plus the V
    bias and S-broadcast.
  sK partition layout: p = 64*par + 4*t + q0, output row h = 2*q0 + par.
"""

import sys

if "/opt/trn_rl_repo" not in sys.path:
    sys.path.insert(0, "/opt/trn_rl_repo")

import numpy as np
import ml_dtypes

BF16 = ml_dtypes.bfloat16

B, C, H, W, S = 2, 64, 32, 32, 16
KS, PAD = 5, 2
NCORES = 8
ROWS = H // 4            # output rows per core
SLAB_R = ROWS + 2 * PAD  # 12
NPAIR = SLAB_R // 2      # 6 slab row-pairs
SLAB_W = W + 2 * PAD     # 36
HW = ROWS * W            # 256
NPO = 3                  # pair-offset classes
M3 = NPO * 2 * KS        # 30 stationary columns (po, dx, par)
NTAP3 = NPO * KS         # 15 surviving tap classes
PLANE = S * 4 * SLAB_W   # 2304: td elements per (po,dx,par) plane
PAIRW = SLAB_W * S       # 576 elements per pair per partition
AW = M3 + PAIRW          # slabA columns: weff3 | pair 0
BW2 = 5 * PAIRW          # slabB columns: pairs 1-5
CST2W = ROWS + ROWS + 128  # hsel | hmask | esel

_MODULE = None


def _build_module():
    import concourse.bacc as bacc
    import concourse.bass as bass
    import concourse.tile as tile
    from concourse import mybir

    f32 = mybir.dt.float32
    bf16 = mybir.dt.bfloat16
    AF = mybir.ActivationFunctionType
    ALU = mybir.AluOpType
    nc = bacc.Bacc("TRN2", target_bir_lowering=False, debug=False, num_devices=NCORES)

    slaba_d = nc.dram_tensor("slaba", [128, AW], bf16, kind="ExternalInput")   # weff | pair0
    slabb_d = nc.dram_tensor("slabb", [128, BW2], bf16, kind="ExternalInput")  # pairs 1-5
    xt_d = nc.dram_tensor("xt", [128, 8, HW], bf16, kind="ExternalInput")
    cst_d = nc.dram_tensor("cst", [128, CST2W], bf16, kind="ExternalInput")
    w1vr_d = nc.dram_tensor("w1vr", [128, 8, C], bf16, kind="ExternalInput")
    o_d = nc.dram_tensor("o", [C, HW], bf16, kind="ExternalOutput")
    dn_d = nc.dram_tensor("dn", [ROWS * W], f32, kind="ExternalOutput")

    # scratch DRAM for the partition-crossing tap gather (po-pre-shifted)
    td_d = nc.dram_tensor("td", [M3, S, 4, SLAB_W], bf16)

    with tile.TileContext(nc) as tc:
        with tc.tile_pool(name="sb", bufs=1) as sb, tc.tile_pool(
            name="ps", bufs=4, space="PSUM"
        ) as ps, tc.tile_pool(name="pso", bufs=1, space="PSUM") as pso:
            # --- PE warm-up: junk matmuls hold the tensor engine p-state
            # while the slab loads (results never read) ---
            s_warm = sb.tile([128, 512], bf16)
            nc.vector.memset(s_warm, 0.125)
            p_w = pso.tile([8, 512], f32, tag="warm")
            for i in range(4):
                nc.tensor.matmul(p_w, s_warm[:, 0:8], s_warm, start=True, stop=True)
            for i in range(3):
                nc.tensor.matmul(
                    p_w[:, 0:64], s_warm[:, 0:8], s_warm[:, 0:64],
                    start=True, stop=True,
                )

            # --- loads: two parallel HBM chains for the slab (+weff3 lead
            # columns on the sync chain), consts behind it, xt on SWDGE ---
            s_sa = sb.tile([128, AW], bf16)
            nc.scalar.dma_start(s_sa, slaba_d.ap())
            s_sb = sb.tile([128, BW2], bf16)
            nc.sync.dma_start(s_sb, slabb_d.ap())
            s_cst = sb.tile([128, CST2W], bf16)
            nc.scalar.dma_start(s_cst, cst_d.ap())
            s_hsel = s_cst[:, 0:ROWS]
            s_hmask = s_cst[:, ROWS : 2 * ROWS]
            s_esel = s_cst[:, 2 * ROWS : 2 * ROWS + 128]
            s_xt = sb.tile([128, 8, HW], bf16)
            nc.gpsimd.dma_start(s_xt, xt_d.ap())
            s_w1vr = sb.tile([128, 8, C], bf16)

            s_weff = s_sa[:, 0:M3]

            def pair_view(q):
                if q == 0:
                    v = s_sa[:, M3 : M3 + PAIRW]
                else:
                    v = s_sb[:, (q - 1) * PAIRW : q * PAIRW]
                return v.rearrange("p (w t) -> p w t", w=SLAB_W)

            # --- phase A: T3[(po,dx,par), (w',t)] = weff3^T @ slab, one
            # matmul per (pair, t-half); transpose copies split across
            # scalar+vector so the copy rate matches the matmul rate ---
            s_T = sb.tile([M3, S, NPAIR, SLAB_W], bf16)
            for q in range(NPAIR):
                pv = pair_view(q)
                for tau in range(2):
                    p_t = ps.tile([M3, SLAB_W, 8], f32, tag="pt")
                    nc.tensor.matmul(
                        p_t, s_weff, pv[:, :, 8 * tau : 8 * tau + 8],
                        start=True, stop=True,
                    )
                    nc.scalar.copy(
                        s_T[:, 8 * tau : 8 * tau + 4, q, :],
                        p_t[:, :, 0:4].transpose([0, 2, 1]),
                    )
                    nc.vector.tensor_copy(
                        s_T[:, 8 * tau + 4 : 8 * tau + 8, q, :],
                        p_t[:, :, 4:8].transpose([0, 2, 1]),
                    )

            # --- T3 to DRAM: 3 po-class writes of po-shifted 4-pair windows,
            # then ONE merged (par,t,q0) gather per po; a partial tap-reduce
            # runs per po as it lands ---
            s_R = sb.tile([128, NTAP3, W], bf16)
            s_par = sb.tile([128, NPO, W], f32)

            def td_write(po, e):
                e.dma_start(
                    td_d.ap()[10 * po : 10 * po + 10],
                    s_T[10 * po : 10 * po + 10, :, po : po + 4, :],
                )

            def gather(po, par, e):
                src = bass.AP(
                    tensor=td_d.ap().tensor,
                    offset=(10 * po + par) * PLANE,
                    ap=[[SLAB_W, 64], [2 * PLANE + 1, KS], [1, W]],
                )
                e.dma_start(
                    s_R[64 * par : 64 * par + 64, KS * po : KS * po + KS, :], src
                )

            def partial(po):
                nc.vector.tensor_reduce(
                    s_par[:, po, :],
                    s_R[:, KS * po : KS * po + KS, :].transpose([0, 2, 1]),
                    axis=mybir.AxisListType.X,
                    op=ALU.add,
                )

            td_write(0, nc.sync)
            td_write(1, nc.sync)
            td_write(2, nc.sync)
            for po in range(NPO):
                gather(po, 0, nc.scalar)
                gather(po, 1, nc.sync)
                partial(po)
            nc.scalar.dma_start(s_w1vr, w1vr_d.ap())

            # --- PE warm-up 2: fires once the po2 gather lands, right before
            # the den/eb/V matmul burst ---
            for i in range(2):
                nc.tensor.matmul(
                    p_w[:, 0:160], s_warm[:, 0:8],
                    s_R[:, 2 * KS : 3 * KS, :], start=True, stop=True,
                )

            # --- final 3-way add of the partials -> sK[(par,t,q0), w] ---
            s_sk = sb.tile([128, W], f32)
            nc.vector.tensor_reduce(
                s_sk, s_par.transpose([0, 2, 1]), axis=mybir.AxisListType.X, op=ALU.add
            )

            # --- e = exp(sK); replicate to the (ci8,t)-partition layout
            # on-chip (critical path); den/recip/dn trail behind it ---
            s_e16 = sb.tile([128, W], bf16)
            nc.scalar.activation(s_e16, s_sk, AF.Exp)
            s_ed = sb.tile([128, ROWS, W], bf16)
            nc.vector.tensor_tensor(
                s_ed,
                s_e16.unsqueeze(1).broadcast_to((128, ROWS, W)),
                s_hmask.unsqueeze(2).broadcast_to((128, ROWS, W)),
                op=ALU.mult,
            )
            p_eb = pso.tile([128, HW], f32, tag="eb")
            nc.tensor.matmul(
                p_eb, s_esel, s_ed.rearrange("p a b -> p (a b)"), start=True, stop=True
            )
            s_eb = sb.tile([128, HW], bf16)
            nc.vector.tensor_copy(s_eb, p_eb)
            p_den = pso.tile([ROWS, W], f32, tag="den")
            nc.tensor.matmul(p_den, s_hsel, s_e16, start=True, stop=True)

            # --- V path: xattn = x_t * e in four chunks; the V matmuls
            # pipeline behind the multiplies ---
            s_xa = sb.tile([128, 8, HW], bf16)
            ebb = s_eb.unsqueeze(1).broadcast_to((128, 2, HW))
            p_o = pso.tile([C, HW], f32, tag="out")
            for qq in range(4):
                nc.vector.tensor_tensor(
                    s_xa[:, 2 * qq : 2 * qq + 2, :],
                    s_xt[:, 2 * qq : 2 * qq + 2, :],
                    ebb,
                    op=ALU.mult,
                )
            for g in range(8):
                nc.tensor.matmul(
                    p_o,
                    s_w1vr[:, g, :],
                    s_xa[:, g, :],
                    start=(g == 0),
                    stop=(g == 7),
                )
            s_rcp = sb.tile([ROWS, W], f32)
            nc.vector.reciprocal(s_rcp, p_den)
            nc.gpsimd.dma_start(dn_d.ap(), s_rcp)
            # raw (unnormalized) output, partition-split copies + dmas so the
            # two rings drain in parallel; host divides by den via dn
            s_o = sb.tile([C, HW], bf16)
            nc.vector.tensor_copy(s_o, p_o)
            nc.sync.dma_start(o_d.ap()[0:32, :], s_o[0:32, :])
            nc.scalar.dma_start(o_d.ap()[32:64, :], s_o[32:64, :])

    nc.compile()
    return nc


def _get_module():
    global _MODULE
    if _MODULE is None:
        _MODULE = _build_module()
    return _MODULE


def make_host_inputs(x, w1, b1, w2, b2):
    """Host-side precompute: folded weights + per-core reflect-padded slices."""
    x = np.ascontiguousarray(np.asarray(x, np.float32))
    w1 = np.asarray(w1, np.float32)
    w2 = np.asarray(w2, np.float32)

    w1K = w1[C : 2 * C, :, 0, 0]          # [c, ci]
    w2K = w2[0, C : 2 * C]                # [c, 5, 5]
    weff = np.einsum("ci,cyx->iyx", w1K, w2K)  # [ci, dy, dx]
    # weff3[(ci,rp), (po,dx,par)] = weff[ci, 2po+rp-par, dx] (OOR dy -> 0)
    weff3 = np.zeros((128, M3), np.float32)
    for rp in range(2):
        for po in range(NPO):
            for par in range(2):
                dy = 2 * po + rp - par
                if 0 <= dy < KS:
                    for dx in range(KS):
                        weff3[2 * np.arange(C) + rp, 10 * po + 2 * dx + par] = weff[
                            :, dy, dx
                        ]
    weff3 = weff3.astype(BF16)
    w1V = w1[2 * C :, :, 0, 0]            # [co, ci]

    # w1vr[(ci8,t), g, co] = w1V[co, 8g+ci8]
    tmp = w1V.T.reshape(8, 8, C)                      # (g, ci8, co)
    w1vr = np.ascontiguousarray(
        np.broadcast_to(tmp[:, :, None, :], (8, 8, S, C))
        .transpose(1, 2, 0, 3)
        .reshape(128, 8, C)
    ).astype(BF16)

    # sK partition p = 64*par + 4*t + q0;  h(p) = 2*q0 + par
    pp = np.arange(128)
    par_p, t_p, q0_p = pp // 64, (pp % 64) // 4, pp % 4
    h_p = 2 * q0_p + par_p

    hsel = np.zeros((128, ROWS), np.float32)
    hsel[pp, h_p] = 1.0
    hsel = hsel.astype(BF16)
    hmask = hsel

    esel = np.zeros((128, 128), np.float32)
    for rep in range(8):
        esel[pp, rep * S + t_p] = 1.0
    esel = esel.astype(BF16)

    cst = np.concatenate([hsel, hmask, esel], axis=1)
    assert cst.shape == (128, CST2W)

    in_maps = []
    for core in range(NCORES):
        b, hc = divmod(core, 4)
        h0 = ROWS * hc
        xp = np.pad(x[b], ((0, 0), (PAD, PAD), (PAD, PAD), (0, 0)), mode="reflect")
        sl = xp[:, h0 : h0 + SLAB_R, :, :]            # [ci, r, w36, t]
        slab = np.ascontiguousarray(
            sl.reshape(C, NPAIR, 2, SLAB_W, S)
            .transpose(0, 2, 1, 3, 4)
            .reshape(128, NPAIR, PAIRW)
        ).astype(BF16)
        slaba = np.ascontiguousarray(
            np.concatenate([weff3, slab[:, 0, :]], axis=1)
        )
        slabb = np.ascontiguousarray(slab[:, 1:, :].reshape(128, BW2))
        xs = x[b][:, h0 : h0 + ROWS, :, :]            # [ci, h, w, t]
        xt = np.ascontiguousarray(
            xs.reshape(8, 8, ROWS, W, S)
            .transpose(1, 4, 0, 2, 3)
            .reshape(128, 8, HW)
        ).astype(BF16)
        in_maps.append(
            {"slaba": slaba, "slabb": slabb, "xt": xt, "cst": cst, "w1vr": w1vr}
        )
    return in_maps


def assemble_output(results, b1):
    b1V = np.asarray(b1, np.float32)[2 * C :]
    out = np.empty((B, C, H, W, S), np.float32)
    for core in range(NCORES):
        b, hc = divmod(core, 4)
        h0 = ROWS * hc
        r = results[core]
        o = r["o"].astype(np.float32).reshape(C, ROWS, W) * r["dn"].reshape(
            1, ROWS, W
        )
        out[b, :, h0 : h0 + ROWS, :, :] = o[:, :, :, None]
    out += b1V[None, :, None, None, None]
    return out


def kernel(x, w1, b1, w2, b2):
    from concourse.bass_utils import run_bass_kernel_spmd

    nc = _get_module()
    in_maps = make_host_inputs(x, w1, b1, w2, b2)
    res = run_bass_kernel_spmd(nc, in_maps, core_ids=list(range(NCORES)))
    return assemble_output(res.results, b1)
